# revision 35
# baseline (speedup 1.0000x reference)
"""Trainium2 Bass kernel for the smoothed Preisach hysteresis model.

Math: the reference per-step update
    s' = where(h_t > h_{t-1}, s + (1-s)*sigmoid((h_t-alpha)/temp),
                              s + (-1-s)*sigmoid((beta-h_t)/temp))
is a first-order linear recurrence. With u = (s+1)/2, the up-mask M_t
(1 if h rose, else 0), z = M - u and dM_t = M_t - M_{t-1}:

    z' = (z + dM_t) * a_t,   a_t = sigmoid(-arg_t)

arg[n,t] = p_t + alpha_n*q_t + beta_n*r_t is computed by a K=6 fp16 PE
matmul with alpha/beta/p split into hi+lo fp16 pairs (fp32 PSUM).

Scan engine: the stock DVE TensorTensorScanArith runs ~2.5 cycles per
element because its (add -> mult) feedback spans two pipeline stages.
TTS2I is a custom DVE uop program that interleaves TWO independent
recurrences element-by-element, so each stream's dependency lands two
cycles back and the pipeline streams at ~1.1 ns/element (measured).
Stream A state feeds back through stage1's a-flop, stream B through the
b-flop (NEXT_ALU_OUT_A/B), making inits robust to pipeline bubbles; two
primer uops latch the per-partition initials from C0/C1.

Layout: the 6 hysteron tiles per core form 3 pairs; each pair's a/dm/z
streams are interleaved along time ([.., 2t]=tile even, [.., 2t+1]=tile
odd). ACT writes sigmoid outputs with stride-2 APs; the readout matmuls
read stride-2 views of z; dm is pair-duplicated on the host.

Readout: m_num_t = sum_n d_n s_tn = d16sum*(2*M_t - 1) - 2*sum_n d_n z_tn
via K=128 fp16 readout matmuls accumulated over the 6 hysteron tiles;
the host applies the affine fixup and epilogue.

Sharding: hysteron dim N=5151 split across 8 cores (644 each, padded to
6 tiles of 128 partitions; padding carries density 0). Each core outputs
its readout partials [1, T]; host reduces across cores.
"""

import copy
import sys

import numpy as np

sys.path.insert(0, "/opt/trn_rl_repo")

N = 5151
T = 4096
TEMP = 0.01
NCORES = 8
P = 128
TILES = 6                 # ceil(644/128); per-core rows padded to 768
NPC = 644                 # hysterons per core (8*644 = 5152 >= N)
ROWS = TILES * P          # 768
K6 = 6                    # arg matmul contraction: a_hi,a_lo,b_hi,b_lo,1,1
PAIRS = TILES // 2        # 3 interleaved tile-pairs
T2 = 2 * T                # interleaved stream length per pair
CH = 512                  # readout chunk along t (one PSUM bank fp32)
ACH = 1024                # ACT/matmul chunk along t (per tile-half)
SCH2 = 2048               # scan chunk (interleaved elems; 1024 t-steps)
NSCH = T2 // SCH2         # 4 waves per pair
SCHT = SCH2 // 2          # t-steps per wave

_PROG_CACHE = {}


# --------------------------------------------------------------------------
# TTS2I: custom DVE uop -- two interleaved tensor_tensor_scans at
# ~1 elem/cycle.
#   out[p,2k]   = stA' = (stA + in0[p,2k])   * in1[p,2k]
#   out[p,2k+1] = stB' = (stB + in0[p,2k+1]) * in1[p,2k+1]
# stA init = s0 (C0, per-partition), stB init = s1 (C1).
# --------------------------------------------------------------------------

def _ref_tts2i(in0, in1, c0, c1, c2):
    d = np.asarray(in0, np.float32)
    a = np.asarray(in1, np.float32)
    Pn = d.shape[0]
    d = d.reshape(Pn, -1)
    a = a.reshape(Pn, -1)
    Nn = d.shape[1]

    def vec(c):
        if isinstance(c, np.ndarray):
            return np.broadcast_to(c.reshape(-1), (Pn,)).astype(
                np.float32).copy()
        return np.full((Pn,), c, np.float32)

    sA, sB = vec(c0), vec(c1)
    out = np.empty((Pn, Nn), np.float32)
    for k in range(Nn // 2):
        sA = (sA + d[:, 2 * k]) * a[:, 2 * k]
        out[:, 2 * k] = sA
        sB = (sB + d[:, 2 * k + 1]) * a[:, 2 * k + 1]
        out[:, 2 * k + 1] = sB
    return out


_TTS2I_CACHE = {}


class _Tts2iOp:
    """Duck-types the DveOp interface consumed by _custom_dve and
    dve_table_for_ops, but compiles to hand-built uops."""

    name = "TTS2I"
    subdim = False

    def __init__(self):
        from concourse.dve_spec import AluOp, C0, C1, Spec, Src0, Src1, scan
        self.spec = Spec(
            body=scan(AluOp.ADD, Src0 * Src1, init=C0) + C1 * Src1,
            reference=_ref_tts2i)

    def compile(self, ver):
        if ver in _TTS2I_CACHE:
            return _TTS2I_CACHE[ver]
        assert ver == "v3", "TTS2I authored for TRN2 (v3) only"
        from concourse import dve_ops
        from concourse.dve_spec import AluOp, C0, Spec, Src0, Src1, lower, scan
        from concourse.dve_uop import (AluInp, DelayInp, DveOpSpec, InpSel,
                                       Trigger)

        base = lower(Spec(body=scan(AluOp.ADD, Src0 * Src1, init=C0)),
                     ver=ver)
        AluOpDp = type(base[0].datapath_config[0].op)

        p1 = copy.deepcopy(base[0])
        p2 = copy.deepcopy(base[0])
        steady_a = copy.deepcopy(base[1])

        for u in (p1, p2, steady_a):
            # add CONST_1 on input lane 4 -> delay lane 3
            u.inp[4] = InpSel.CONST_1
            u.inp_enable[4] = 1
            for blk in u.datapath_config:
                blk.delay[3] = DelayInp.PREV_DELAY
                blk.delay_enable[3] = 1

        # Primers: P1 latches C0 into stage1's a-flop, P2 latches C1 into
        # the b-flop. Separate flops survive pipeline bubbles between the
        # primers and the first data elements.
        for p, lane, a_en, b_en in (
            (p1, AluInp.PREV_DELAY_2, 1, 0),
            (p2, AluInp.PREV_DELAY_3, 0, 1),
        ):
            s0b = p.datapath_config[0]
            s0b.op = AluOpDp.BYPASS
            s0b.alu_src0 = AluInp.PREV_DELAY_0
            s0b.alu_src1 = AluInp.PREV_DELAY_0
            s1b = p.datapath_config[1]
            s1b.op = AluOpDp.BYPASS
            s1b.alu_src0 = lane
            s1b.alu_src1 = lane
            s1b.alu_out_enable = 1
            s1b.alu_out_a_enable = a_en
            s1b.alu_out_b_enable = b_en
        p1.trigger = (Trigger.COUNT, Trigger.NONE, Trigger.NONE)
        p1.repeat_count = 1
        p1.next_uop = (1, 0, 0)
        p2.trigger = (Trigger.COUNT, Trigger.NONE, Trigger.NONE)
        p2.repeat_count = 1
        p2.next_uop = (2, 0, 0)

        # Steady state: alternating uops; stream A reads/writes the a-flop,
        # stream B the b-flop.
        steady_b = copy.deepcopy(steady_a)
        for u, flop, a_en, b_en, nxt in (
            (steady_a, AluInp.NEXT_ALU_OUT_A, 1, 0, 3),
            (steady_b, AluInp.NEXT_ALU_OUT_B, 0, 1, 2),
        ):
            s0b = u.datapath_config[0]
            s0b.op = AluOpDp.ADD
            s0b.alu_src0 = AluInp.PREV_DELAY_0
            s0b.alu_src1 = flop
            s1b = u.datapath_config[1]
            s1b.op = AluOpDp.MULTIPLY
            s1b.alu_src0 = AluInp.PREV_ALU_OUT
            s1b.alu_src1 = AluInp.PREV_DELAY_1
            s1b.alu_out_enable = 1
            s1b.alu_out_a_enable = a_en
            s1b.alu_out_b_enable = b_en
            u.trigger = (Trigger.SRC_TENSOR_DONE, Trigger.COUNT, Trigger.NONE)
            u.repeat_count = 1
            u.next_uop = (0, nxt, 0)

        spec = DveOpSpec(
            name=self.name,
            opcode=dve_ops.get_dve_sub_opcode(self.name),
            uops=[p1, p2, steady_a, steady_b],
            rd1_en=True,
        )
        _TTS2I_CACHE[ver] = spec
        return spec


def _register_tts2i():
    from concourse import dve_ops
    if "TTS2I" in dve_ops._SUB_OPCODE_FOR_NAME:
        return next(o for o in dve_ops.OPS if o.name == "TTS2I")
    op = _Tts2iOp()
    row = max(dve_ops._SUB_OPCODE_FOR_NAME.values()) + 1
    assert row < 0x20
    dve_ops._SUB_OPCODE_FOR_NAME["TTS2I"] = row
    dve_ops.OPS.append(op)
    dve_ops.CUSTOM_DVE_SPECS["TTS2I"] = op.spec
    return op


# --------------------------------------------------------------------------
# Program
# --------------------------------------------------------------------------

def _build_program(reps=1, loop_n=0, skip=()):
    import contextlib

    import concourse.bass as bass
    import concourse.tile as tile
    from concourse import bacc, mybir

    tts2i = _register_tts2i()

    f32 = mybir.dt.float32
    f16 = mybir.dt.float16
    nc = bacc.Bacc("TRN2", target_bir_lowering=False, debug=False,
                   num_devices=NCORES)

    wt_d = nc.dram_tensor("wt", [K6, ROWS], f16, kind="ExternalInput")
    v_d = nc.dram_tensor("v", [K6, T], f16, kind="ExternalInput")
    dm_d = nc.dram_tensor("dm", [T], f16, kind="ExternalInput")
    dens_d = nc.dram_tensor("dens", [P, TILES], f16, kind="ExternalInput")
    mpart_d = nc.dram_tensor("mpart", [1, T], f32, kind="ExternalOutput")

    wt_ap = wt_d.ap()
    v_ap = v_d.ap()
    dm_ap = dm_d.ap()
    dens_ap = dens_d.ap()
    mpart_ap = mpart_d.ap()

    ts = bass.ts
    Sigmoid = mybir.ActivationFunctionType.Sigmoid

    def strided(base, off, step, cnt):
        # [128, cnt] view of `base` (a full-tile AP) at inner offset `off`
        # with inner stride `step`
        return bass.AP(tensor=base.tensor, offset=base.offset + off,
                       ap=[list(base.ap[0]), [step, cnt]])

    with tile.TileContext(nc) as tc:
        from contextlib import ExitStack
        with ExitStack() as ctx:
            consts = ctx.enter_context(tc.tile_pool(name="consts", bufs=1))
            apool = ctx.enter_context(tc.tile_pool(name="a", bufs=3))
            zpool = ctx.enter_context(tc.tile_pool(name="z", bufs=PAIRS))
            mpool = ctx.enter_context(tc.tile_pool(name="m", bufs=2))
            ps_arg = ctx.enter_context(
                tc.tile_pool(name="ps_arg", bufs=3, space="PSUM"))
            ps_m = ctx.enter_context(
                tc.tile_pool(name="ps_m", bufs=2, space="PSUM"))

            wt_sb = consts.tile([K6, ROWS], f16)
            v_sb = consts.tile([K6, T], f16)
            dens_sb = consts.tile([P, TILES], f16)
            dm_bc = consts.tile([P, T], f16)
            ini32 = consts.tile([P, 2 * PAIRS], f32)

            nc.sync.dma_start(out=wt_sb[:], in_=wt_ap[:, :])
            nc.sync.dma_start(out=v_sb[:], in_=v_ap[:, :])
            # broadcast the dM row to all 128 partitions via 0-stride DMA,
            # chunked across queues; the scan reads it pair-duplicated
            # through a stride-0 inner AP dim
            for j in range(8):
                c = T // 8
                src = bass.AP(tensor=dm_ap.tensor,
                              offset=dm_ap.offset + j * c,
                              ap=[[0, P], [1, c]])
                nc.sync.dma_start(out=dm_bc[:, ts(j, c)], in_=src)
            nc.sync.dma_start(out=dens_sb[:], in_=dens_ap[:, :])

            if loop_n:
                loop_cm = tc.For_i(
                    0, loop_n, 1,
                    hint_engines=(mybir.EngineType.PE,
                                  mybir.EngineType.Activation,
                                  mybir.EngineType.DVE,
                                  mybir.EngineType.Pool))
            else:
                loop_cm = contextlib.nullcontext()
            with loop_cm:
              for _rep in range(reps):
                z_tiles = [zpool.tile([P, T2], f16, name=f"z{p}")
                           for p in range(PAIRS)]

                def emit_readout(c):
                    # readout for the t-range covered by wave c; emitted one
                    # wave late so the in-order PE queue never stalls the
                    # next wave's arg matmuls behind scan-dependent readouts
                    m_sb = mpool.tile([1, SCHT], f32, tag="m")
                    for jj in range(SCHT // CH):
                        t0 = c * SCHT + jj * CH
                        mp = ps_m.tile([1, CH], f32)
                        for p in range(PAIRS):
                            for half in range(2):
                                i = 2 * p + half
                                h0 = half * T + t0
                                nc.tensor.matmul(
                                    out=mp[:],
                                    lhsT=dens_sb[:, i:i + 1],
                                    rhs=z_tiles[p][:, h0:h0 + CH],
                                    start=(p == 0 and half == 0),
                                    stop=(p == PAIRS - 1 and half == 1),
                                )
                        nc.scalar.copy(out=m_sb[:, ts(jj, CH)], in_=mp[:])
                        nc.sync.dma_start(
                            out=mpart_ap[:, t0:t0 + CH],
                            in_=m_sb[:, ts(jj, CH)])

                for c in range(NSCH):
                    for p in range(PAIRS):
                        # a-tile holds the two halves CONTIGUOUSLY
                        # ([even-tile a | odd-tile a]); the scan interleaves
                        # them through a rank-3 AP so ACT writes stay packed
                        a = apool.tile([P, SCH2], f16)
                        for half in range(2):
                            i = 2 * p + half       # hysteron tile index
                            for aj in range(SCHT // ACH):
                                arg = ps_arg.tile([P, ACH], f32, tag="arg")
                                for jj in range(ACH // CH):
                                    j0 = c * SCHT + aj * ACH + jj * CH
                                    nc.tensor.matmul(
                                        out=arg[:, ts(jj, CH)],
                                        lhsT=wt_sb[:, ts(i, P)],
                                        rhs=v_sb[:, j0:j0 + CH],
                                        start=True, stop=True,
                                    )
                                # a stays interleaved (strided ACT write):
                                # a 3D in1 would demote s1 to a literal,
                                # breaking the chunk-chained iniB AP
                                dst = strided(a[:], 2 * aj * ACH + half,
                                              2, ACH)
                                nc.scalar.activation(
                                    out=dst, in_=arg[:],
                                    func=Sigmoid, scale=-1.0)
                        if "scan" not in skip:
                            zb = z_tiles[p][:]
                            if c == 0:
                                iniA, iniB = 0.0, 0.0
                            else:
                                # ISA wants fp32 scalar APs; stage the two
                                # region-end z values through a fp32 copy
                                ini_src = bass.AP(
                                    tensor=zb.tensor,
                                    offset=zb.offset + c * SCHT - 1,
                                    ap=[list(zb.ap[0]), [T, 2]])
                                nc.vector.tensor_copy(
                                    out=ini32[:, 2 * p:2 * p + 2],
                                    in_=ini_src)
                                iniA = ini32[:, 2 * p:2 * p + 1]
                                iniB = ini32[:, 2 * p + 1:2 * p + 2]
                            dmb = dm_bc[:]
                            dm_in = bass.AP(
                                tensor=dmb.tensor,
                                offset=dmb.offset + c * SCHT,
                                ap=[list(dmb.ap[0]), [1, SCHT], [0, 2]])
                            a_in = a[:]
                            # z layout: [even-tile all T | odd-tile all T];
                            # the rank-3 out AP de-interleaves the streams
                            z_out = bass.AP(
                                tensor=zb.tensor,
                                offset=zb.offset + c * SCHT,
                                ap=[list(zb.ap[0]), [1, SCHT], [T, 2]])
                            nc.vector._custom_dve(
                                tts2i,
                                out=z_out,
                                in0=dm_in,
                                in1=a_in,
                                s0=iniA, s1=iniB)
                        else:
                            nc.vector.tensor_copy(
                                out=z_tiles[p][:, ts(c, SCH2)], in_=a[:])

                    if c >= 1:
                        emit_readout(c - 1)
                emit_readout(NSCH - 1)
    nc.compile()
    return nc


def _split16(x):
    hi = x.astype(np.float16)
    lo = (x - hi.astype(np.float64)).astype(np.float16)
    return hi, lo


def _host_prep(h, mesh_points, raw_density):
    h = np.asarray(h, np.float32)
    mesh = np.asarray(mesh_points, np.float32)
    rd = np.asarray(raw_density, np.float32)
    beta = mesh[:, 0].astype(np.float64)
    alpha = mesh[:, 1].astype(np.float64)

    hprev = np.concatenate([[np.float32(0.0)], h[:-1]])
    up = h > hprev
    R = np.float64(1.0) / np.float64(np.float32(TEMP))
    h64 = h.astype(np.float64)
    q = np.where(up, -R, 0.0)
    r = np.where(up, 0.0, R)
    p = np.where(up, R * h64, -R * h64)
    p_hi, p_lo = _split16(p)
    q16 = q.astype(np.float16)
    r16 = r.astype(np.float16)
    V6 = np.stack([q16, q16, r16, r16, p_hi, p_lo]).astype(np.float16)

    M = up.astype(np.float64)                 # M_t in {0,1}
    Mprev = np.concatenate([[0.0], M[:-1]])
    dM = (M - Mprev).astype(np.float16)       # in {-1,0,1}

    dens = (1.0 / (1.0 + np.exp(-rd.astype(np.float64))))  # [N] float64

    pad = NCORES * NPC - N   # 1
    alpha_p = np.concatenate([alpha, np.full(pad, 0.5)])
    beta_p = np.concatenate([beta, np.full(pad, 0.5)])
    dens_p = np.concatenate([dens, np.zeros(pad)])

    in_maps = []
    d16sum = 0.0
    for c in range(NCORES):
        sl = slice(c * NPC, (c + 1) * NPC)
        a_c = np.full(ROWS, 0.5)
        b_c = np.full(ROWS, 0.5)
        d_c = np.zeros(ROWS)
        a_c[:NPC] = alpha_p[sl]
        b_c[:NPC] = beta_p[sl]
        d_c[:NPC] = dens_p[sl]
        ah, al = _split16(a_c)
        bh, bl = _split16(b_c)
        wt = np.stack([ah, al, bh, bl,
                       np.ones(ROWS, np.float16),
                       np.ones(ROWS, np.float16)]).astype(np.float16)
        dens16 = d_c.astype(np.float16)
        dens_tiles = dens16.reshape(TILES, P).T  # [P, TILES]
        d16sum += dens16.astype(np.float64).sum()
        in_maps.append({
            "wt": wt,
            "v": V6,
            "dm": dM,
            "dens": dens_tiles,
        })
    return in_maps, dens, h, d16sum, M


def kernel(h, mesh_points, raw_density, raw_offset, raw_scale, raw_slope):
    from concourse.bass_utils import run_bass_kernel_spmd

    in_maps, dens, h32, d16sum, M = _host_prep(h, mesh_points, raw_density)

    if "prog" not in _PROG_CACHE:
        _PROG_CACHE["prog"] = _build_program()
    nc = _PROG_CACHE["prog"]

    res = run_bass_kernel_spmd(nc, in_maps, list(range(NCORES)))
    zpart = np.zeros(T, np.float64)
    for c in range(NCORES):
        zpart += res.results[c]["mpart"].astype(np.float64).reshape(T)

    def sigm(x):
        return 1.0 / (1.0 + np.exp(-np.float64(np.asarray(x, np.float32)[0])))

    offset = -10.0 + 20.0 * sigm(raw_offset)
    scale = 20.0 * sigm(raw_scale)
    slope = -20.0 + 40.0 * sigm(raw_slope)

    # s = 2u-1, u = M - z  =>  sum(d*s) = d16sum*(2M-1) - 2*sum(d*z)
    m = (d16sum * (2.0 * M - 1.0) - 2.0 * zpart) / dens.sum()
    out = scale * m + h32.astype(np.float64) * slope + offset
    return out.astype(np.float32)


# revision 36
# speedup vs baseline: 3.1136x; 3.1136x over previous
"""Trainium2 Bass kernel for the smoothed Preisach hysteresis model.

Math: the reference per-step update
    s' = where(h_t > h_{t-1}, s + (1-s)*sigmoid((h_t-alpha)/temp),
                              s + (-1-s)*sigmoid((beta-h_t)/temp))
is a first-order linear recurrence. With u = (s+1)/2, the up-mask M_t
(1 if h rose, else 0), z = M - u and dM_t = M_t - M_{t-1}:

    z' = (z + dM_t) * a_t,   a_t = sigmoid(-arg_t)

arg[n,t] = p_t + alpha_n*q_t + beta_n*r_t is computed by a K=6 fp16 PE
matmul with alpha/beta/p split into hi+lo fp16 pairs (fp32 PSUM).

Scan engine: the stock DVE TensorTensorScanArith runs ~2.5 cycles per
element because its (add -> mult) feedback spans two pipeline stages.
TTS2I is a custom DVE uop program that interleaves TWO independent
recurrences element-by-element, so each stream's dependency lands two
cycles back and the pipeline streams at ~1.1 ns/element (measured).
Stream A state feeds back through stage1's a-flop, stream B through the
b-flop (NEXT_ALU_OUT_A/B), making inits robust to pipeline bubbles; two
primer uops latch the per-partition initials from C0/C1.

Layout: the 6 hysteron tiles per core form 3 pairs; each pair's a/dm/z
streams are interleaved along time ([.., 2t]=tile even, [.., 2t+1]=tile
odd). ACT writes sigmoid outputs with stride-2 APs; the readout matmuls
read stride-2 views of z; dm is pair-duplicated on the host.

Readout: m_num_t = sum_n d_n s_tn = d16sum*(2*M_t - 1) - 2*sum_n d_n z_tn
via K=128 fp16 readout matmuls accumulated over the 6 hysteron tiles;
the host applies the affine fixup and epilogue.

Sharding: hysteron dim N=5151 split across 8 cores (644 each, padded to
6 tiles of 128 partitions; padding carries density 0). Each core outputs
its readout partials [1, T]; host reduces across cores.
"""

import copy
import sys

import numpy as np

sys.path.insert(0, "/opt/trn_rl_repo")

N = 5151
T = 4096
TEMP = 0.01
NCORES = 8
P = 128
TILES = 6                 # ceil(644/128); per-core rows padded to 768
NPC = 644                 # hysterons per core (8*644 = 5152 >= N)
ROWS = TILES * P          # 768
K6 = 6                    # arg matmul contraction: a_hi,a_lo,b_hi,b_lo,1,1
PAIRS = TILES // 2        # 3 interleaved tile-pairs
T2 = 2 * T                # interleaved stream length per pair
CH = 512                  # readout chunk along t (one PSUM bank fp32)
ACH = 1024                # ACT/matmul chunk along t (per tile-half)
SCH2 = 2048               # scan chunk (interleaved elems; 1024 t-steps)
NSCH = T2 // SCH2         # 4 waves per pair
SCHT = SCH2 // 2          # t-steps per wave

_PROG_CACHE = {}


# --------------------------------------------------------------------------
# TTS2I: custom DVE uop -- two interleaved tensor_tensor_scans at
# ~1 elem/cycle.
#   out[p,2k]   = stA' = (stA + in0[p,2k])   * in1[p,2k]
#   out[p,2k+1] = stB' = (stB + in0[p,2k+1]) * in1[p,2k+1]
# stA init = s0 (C0, per-partition), stB init = s1 (C1).
# --------------------------------------------------------------------------

def _ref_tts2i(in0, in1, c0, c1, c2):
    d = np.asarray(in0, np.float32)
    a = np.asarray(in1, np.float32)
    Pn = d.shape[0]
    d = d.reshape(Pn, -1)
    a = a.reshape(Pn, -1)
    Nn = d.shape[1]

    def vec(c):
        if isinstance(c, np.ndarray):
            return np.broadcast_to(c.reshape(-1), (Pn,)).astype(
                np.float32).copy()
        return np.full((Pn,), c, np.float32)

    sA, sB = vec(c0), vec(c1)
    out = np.empty((Pn, Nn), np.float32)
    for k in range(Nn // 2):
        sA = (sA + d[:, 2 * k]) * a[:, 2 * k]
        out[:, 2 * k] = sA
        sB = (sB + d[:, 2 * k + 1]) * a[:, 2 * k + 1]
        out[:, 2 * k + 1] = sB
    return out


_TTS2I_CACHE = {}


class _Tts2iOp:
    """Duck-types the DveOp interface consumed by _custom_dve and
    dve_table_for_ops, but compiles to hand-built uops."""

    name = "TTS2I"
    subdim = False

    def __init__(self):
        from concourse.dve_spec import AluOp, C0, C1, Spec, Src0, Src1, scan
        self.spec = Spec(
            body=scan(AluOp.ADD, Src0 * Src1, init=C0) + C1 * Src1,
            reference=_ref_tts2i)

    def compile(self, ver):
        if ver in _TTS2I_CACHE:
            return _TTS2I_CACHE[ver]
        assert ver == "v3", "TTS2I authored for TRN2 (v3) only"
        from concourse import dve_ops
        from concourse.dve_spec import AluOp, C0, Spec, Src0, Src1, lower, scan
        from concourse.dve_uop import (AluInp, DelayInp, DveOpSpec, InpSel,
                                       Trigger)

        base = lower(Spec(body=scan(AluOp.ADD, Src0 * Src1, init=C0)),
                     ver=ver)
        AluOpDp = type(base[0].datapath_config[0].op)

        p1 = copy.deepcopy(base[0])
        p2 = copy.deepcopy(base[0])
        steady_a = copy.deepcopy(base[1])

        for u in (p1, p2, steady_a):
            # add CONST_1 on input lane 4 -> delay lane 3
            u.inp[4] = InpSel.CONST_1
            u.inp_enable[4] = 1
            for blk in u.datapath_config:
                blk.delay[3] = DelayInp.PREV_DELAY
                blk.delay_enable[3] = 1

        # Primers: P1 latches C0 into stage1's a-flop, P2 latches C1 into
        # the b-flop. Separate flops survive pipeline bubbles between the
        # primers and the first data elements.
        for p, lane, a_en, b_en in (
            (p1, AluInp.PREV_DELAY_2, 1, 0),
            (p2, AluInp.PREV_DELAY_3, 0, 1),
        ):
            s0b = p.datapath_config[0]
            s0b.op = AluOpDp.BYPASS
            s0b.alu_src0 = AluInp.PREV_DELAY_0
            s0b.alu_src1 = AluInp.PREV_DELAY_0
            s1b = p.datapath_config[1]
            s1b.op = AluOpDp.BYPASS
            s1b.alu_src0 = lane
            s1b.alu_src1 = lane
            s1b.alu_out_enable = 1
            s1b.alu_out_a_enable = a_en
            s1b.alu_out_b_enable = b_en
        p1.trigger = (Trigger.COUNT, Trigger.NONE, Trigger.NONE)
        p1.repeat_count = 1
        p1.next_uop = (1, 0, 0)
        p2.trigger = (Trigger.COUNT, Trigger.NONE, Trigger.NONE)
        p2.repeat_count = 1
        p2.next_uop = (2, 0, 0)

        # Steady state: alternating uops; stream A reads/writes the a-flop,
        # stream B the b-flop.
        steady_b = copy.deepcopy(steady_a)
        for u, flop, a_en, b_en, nxt in (
            (steady_a, AluInp.NEXT_ALU_OUT_A, 1, 0, 3),
            (steady_b, AluInp.NEXT_ALU_OUT_B, 0, 1, 2),
        ):
            s0b = u.datapath_config[0]
            s0b.op = AluOpDp.ADD
            s0b.alu_src0 = AluInp.PREV_DELAY_0
            s0b.alu_src1 = flop
            s1b = u.datapath_config[1]
            s1b.op = AluOpDp.MULTIPLY
            s1b.alu_src0 = AluInp.PREV_ALU_OUT
            s1b.alu_src1 = AluInp.PREV_DELAY_1
            s1b.alu_out_enable = 1
            s1b.alu_out_a_enable = a_en
            s1b.alu_out_b_enable = b_en
            u.trigger = (Trigger.SRC_TENSOR_DONE, Trigger.COUNT, Trigger.NONE)
            u.repeat_count = 1
            u.next_uop = (0, nxt, 0)

        spec = DveOpSpec(
            name=self.name,
            opcode=dve_ops.get_dve_sub_opcode(self.name),
            uops=[p1, p2, steady_a, steady_b],
            rd1_en=True,
        )
        _TTS2I_CACHE[ver] = spec
        return spec


def _register_tts2i():
    from concourse import dve_ops
    if "TTS2I" in dve_ops._SUB_OPCODE_FOR_NAME:
        return next(o for o in dve_ops.OPS if o.name == "TTS2I")
    op = _Tts2iOp()
    row = max(dve_ops._SUB_OPCODE_FOR_NAME.values()) + 1
    assert row < 0x20
    dve_ops._SUB_OPCODE_FOR_NAME["TTS2I"] = row
    dve_ops.OPS.append(op)
    dve_ops.CUSTOM_DVE_SPECS["TTS2I"] = op.spec
    return op


# --------------------------------------------------------------------------
# Program
# --------------------------------------------------------------------------

def _build_program(reps=1, loop_n=0, skip=()):
    import contextlib

    import concourse.bass as bass
    import concourse.tile as tile
    from concourse import bacc, mybir

    tts2i = _register_tts2i()

    f32 = mybir.dt.float32
    f16 = mybir.dt.float16
    nc = bacc.Bacc("TRN2", target_bir_lowering=False, debug=False,
                   num_devices=NCORES)

    wt_d = nc.dram_tensor("wt", [K6, ROWS], f16, kind="ExternalInput")
    v_d = nc.dram_tensor("v", [K6, T], f16, kind="ExternalInput")
    dm_d = nc.dram_tensor("dm", [T], f16, kind="ExternalInput")
    dens_d = nc.dram_tensor("dens", [P, TILES], f16, kind="ExternalInput")
    mpart_d = nc.dram_tensor("mpart", [1, T], f32, kind="ExternalOutput")

    wt_ap = wt_d.ap()
    v_ap = v_d.ap()
    dm_ap = dm_d.ap()
    dens_ap = dens_d.ap()
    mpart_ap = mpart_d.ap()

    ts = bass.ts
    Sigmoid = mybir.ActivationFunctionType.Sigmoid

    def strided(base, off, step, cnt):
        # [128, cnt] view of `base` (a full-tile AP) at inner offset `off`
        # with inner stride `step`
        return bass.AP(tensor=base.tensor, offset=base.offset + off,
                       ap=[list(base.ap[0]), [step, cnt]])

    with tile.TileContext(nc) as tc:
        from contextlib import ExitStack
        with ExitStack() as ctx:
            consts = ctx.enter_context(tc.tile_pool(name="consts", bufs=1))
            apool = ctx.enter_context(tc.tile_pool(name="a", bufs=3))
            zpool = ctx.enter_context(tc.tile_pool(name="z", bufs=PAIRS))
            mpool = ctx.enter_context(tc.tile_pool(name="m", bufs=2))
            ps_arg = ctx.enter_context(
                tc.tile_pool(name="ps_arg", bufs=3, space="PSUM"))
            ps_m = ctx.enter_context(
                tc.tile_pool(name="ps_m", bufs=2, space="PSUM"))

            wt_sb = consts.tile([K6, ROWS], f16)
            v_sb = consts.tile([K6, T], f16)
            dens_sb = consts.tile([P, TILES], f16)
            dm_bc = consts.tile([P, T], f16)
            ini32 = consts.tile([P, 2 * PAIRS], f32)

            nc.sync.dma_start(out=wt_sb[:], in_=wt_ap[:, :])
            nc.sync.dma_start(out=v_sb[:], in_=v_ap[:, :])
            # broadcast the dM row to all 128 partitions via 0-stride DMA,
            # chunked across queues; the scan reads it pair-duplicated
            # through a stride-0 inner AP dim
            for j in range(8):
                c = T // 8
                src = bass.AP(tensor=dm_ap.tensor,
                              offset=dm_ap.offset + j * c,
                              ap=[[0, P], [1, c]])
                nc.sync.dma_start(out=dm_bc[:, ts(j, c)], in_=src)
            nc.sync.dma_start(out=dens_sb[:], in_=dens_ap[:, :])

            if loop_n:
                loop_cm = tc.For_i(
                    0, loop_n, 1,
                    hint_engines=(mybir.EngineType.PE,
                                  mybir.EngineType.Activation,
                                  mybir.EngineType.DVE,
                                  mybir.EngineType.Pool))
            else:
                loop_cm = contextlib.nullcontext()
            with loop_cm:
              for _rep in range(reps):
                z_tiles = [zpool.tile([P, T2], f16, name=f"z{p}")
                           for p in range(PAIRS)]

                def emit_readout(c):
                    # readout for the t-range covered by wave c; emitted one
                    # wave late so the in-order PE queue never stalls the
                    # next wave's arg matmuls behind scan-dependent readouts
                    m_sb = mpool.tile([1, SCHT], f32, tag="m")
                    for jj in range(SCHT // CH):
                        t0 = c * SCHT + jj * CH
                        mp = ps_m.tile([1, CH], f32)
                        for p in range(PAIRS):
                            zbase = z_tiles[p][:]
                            for half in range(2):
                                i = 2 * p + half
                                rhs = strided(zbase, 2 * t0 + half, 2, CH)
                                nc.tensor.matmul(
                                    out=mp[:],
                                    lhsT=dens_sb[:, i:i + 1],
                                    rhs=rhs,
                                    start=(p == 0 and half == 0),
                                    stop=(p == PAIRS - 1 and half == 1),
                                )
                        nc.scalar.copy(out=m_sb[:, ts(jj, CH)], in_=mp[:])
                        nc.sync.dma_start(
                            out=mpart_ap[:, t0:t0 + CH],
                            in_=m_sb[:, ts(jj, CH)])

                for c in range(NSCH):
                    for p in range(PAIRS):
                        a = apool.tile([P, SCH2], f16)
                        for half in range(2):
                            i = 2 * p + half       # hysteron tile index
                            for aj in range(SCHT // ACH):
                                arg = ps_arg.tile([P, ACH], f32, tag="arg")
                                for jj in range(ACH // CH):
                                    j0 = c * SCHT + aj * ACH + jj * CH
                                    nc.tensor.matmul(
                                        out=arg[:, ts(jj, CH)],
                                        lhsT=wt_sb[:, ts(i, P)],
                                        rhs=v_sb[:, j0:j0 + CH],
                                        start=True, stop=True,
                                    )
                                # a[2k+half] = sigmoid(-arg), strided write
                                dst = strided(a[:], 2 * aj * ACH + half,
                                              2, ACH)
                                nc.scalar.activation(
                                    out=dst, in_=arg[:],
                                    func=Sigmoid, scale=-1.0)
                        if "scan" not in skip:
                            off = c * SCH2
                            if c == 0:
                                iniA, iniB = 0.0, 0.0
                            else:
                                # ISA wants fp32 scalar APs; stage the z
                                # chain values through a tiny fp32 copy
                                nc.vector.tensor_copy(
                                    out=ini32[:, 2 * p:2 * p + 2],
                                    in_=z_tiles[p][:, off - 2:off])
                                iniA = ini32[:, 2 * p:2 * p + 1]
                                iniB = ini32[:, 2 * p + 1:2 * p + 2]
                            dmb = dm_bc[:]
                            dm_in = bass.AP(
                                tensor=dmb.tensor,
                                offset=dmb.offset + c * SCHT,
                                ap=[list(dmb.ap[0]), [1, SCHT], [0, 2]])
                            nc.vector._custom_dve(
                                tts2i,
                                out=z_tiles[p][:, ts(c, SCH2)],
                                in0=dm_in,
                                in1=a[:],
                                s0=iniA, s1=iniB)
                        else:
                            nc.vector.tensor_copy(
                                out=z_tiles[p][:, ts(c, SCH2)], in_=a[:])

                    if c >= 1:
                        emit_readout(c - 1)
                emit_readout(NSCH - 1)
    nc.compile()
    return nc


def _split16(x):
    hi = x.astype(np.float16)
    lo = (x - hi.astype(np.float64)).astype(np.float16)
    return hi, lo


def _host_prep(h, mesh_points, raw_density):
    h = np.asarray(h, np.float32)
    mesh = np.asarray(mesh_points, np.float32)
    rd = np.asarray(raw_density, np.float32)
    beta = mesh[:, 0].astype(np.float64)
    alpha = mesh[:, 1].astype(np.float64)

    hprev = np.concatenate([[np.float32(0.0)], h[:-1]])
    up = h > hprev
    R = np.float64(1.0) / np.float64(np.float32(TEMP))
    h64 = h.astype(np.float64)
    q = np.where(up, -R, 0.0)
    r = np.where(up, 0.0, R)
    p = np.where(up, R * h64, -R * h64)
    p_hi, p_lo = _split16(p)
    q16 = q.astype(np.float16)
    r16 = r.astype(np.float16)
    V6 = np.stack([q16, q16, r16, r16, p_hi, p_lo]).astype(np.float16)

    M = up.astype(np.float64)                 # M_t in {0,1}
    Mprev = np.concatenate([[0.0], M[:-1]])
    dM = (M - Mprev).astype(np.float16)       # in {-1,0,1}

    dens = (1.0 / (1.0 + np.exp(-rd.astype(np.float64))))  # [N] float64

    pad = NCORES * NPC - N   # 1
    alpha_p = np.concatenate([alpha, np.full(pad, 0.5)])
    beta_p = np.concatenate([beta, np.full(pad, 0.5)])
    dens_p = np.concatenate([dens, np.zeros(pad)])

    in_maps = []
    d16sum = 0.0
    for c in range(NCORES):
        sl = slice(c * NPC, (c + 1) * NPC)
        a_c = np.full(ROWS, 0.5)
        b_c = np.full(ROWS, 0.5)
        d_c = np.zeros(ROWS)
        a_c[:NPC] = alpha_p[sl]
        b_c[:NPC] = beta_p[sl]
        d_c[:NPC] = dens_p[sl]
        ah, al = _split16(a_c)
        bh, bl = _split16(b_c)
        wt = np.stack([ah, al, bh, bl,
                       np.ones(ROWS, np.float16),
                       np.ones(ROWS, np.float16)]).astype(np.float16)
        dens16 = d_c.astype(np.float16)
        dens_tiles = dens16.reshape(TILES, P).T  # [P, TILES]
        d16sum += dens16.astype(np.float64).sum()
        in_maps.append({
            "wt": wt,
            "v": V6,
            "dm": dM,
            "dens": dens_tiles,
        })
    return in_maps, dens, h, d16sum, M


def kernel(h, mesh_points, raw_density, raw_offset, raw_scale, raw_slope):
    from concourse.bass_utils import run_bass_kernel_spmd

    in_maps, dens, h32, d16sum, M = _host_prep(h, mesh_points, raw_density)

    if "prog" not in _PROG_CACHE:
        _PROG_CACHE["prog"] = _build_program()
    nc = _PROG_CACHE["prog"]

    res = run_bass_kernel_spmd(nc, in_maps, list(range(NCORES)))
    zpart = np.zeros(T, np.float64)
    for c in range(NCORES):
        zpart += res.results[c]["mpart"].astype(np.float64).reshape(T)

    def sigm(x):
        return 1.0 / (1.0 + np.exp(-np.float64(np.asarray(x, np.float32)[0])))

    offset = -10.0 + 20.0 * sigm(raw_offset)
    scale = 20.0 * sigm(raw_scale)
    slope = -20.0 + 40.0 * sigm(raw_slope)

    # s = 2u-1, u = M - z  =>  sum(d*s) = d16sum*(2M-1) - 2*sum(d*z)
    m = (d16sum * (2.0 * M - 1.0) - 2.0 * zpart) / dens.sum()
    out = scale * m + h32.astype(np.float64) * slope + offset
    return out.astype(np.float32)


# revision 37
# speedup vs baseline: 4.0535x; 1.3019x over previous
"""Trainium2 Bass kernel for the smoothed Preisach hysteresis model.

Math: the reference per-step update
    s' = where(h_t > h_{t-1}, s + (1-s)*sigmoid((h_t-alpha)/temp),
                              s + (-1-s)*sigmoid((beta-h_t)/temp))
is a first-order linear recurrence. With u = (s+1)/2, the up-mask M_t
(1 if h rose, else 0), z = M - u and dM_t = M_t - M_{t-1}:

    z' = (z + dM_t) * a_t,   a_t = sigmoid(-arg_t)

arg[n,t] = p_t + alpha_n*q_t + beta_n*r_t is computed by a K=6 fp16 PE
matmul with alpha/beta/p split into hi+lo fp16 pairs (fp32 PSUM).

Scan engine: the stock DVE TensorTensorScanArith runs ~2.5 cycles per
element because its (add -> mult) feedback spans two pipeline stages.
TTS2I is a custom DVE uop program that interleaves TWO independent
recurrences element-by-element, so each stream's dependency lands two
cycles back and the pipeline streams at ~1.1 ns/element (measured).
Stream A state feeds back through stage1's a-flop, stream B through the
b-flop (NEXT_ALU_OUT_A/B), making inits robust to pipeline bubbles; two
primer uops latch the per-partition initials from C0/C1.

Layout: the 6 hysteron tiles per core form 3 pairs; each pair's a/dm/z
streams are interleaved along time ([.., 2t]=tile even, [.., 2t+1]=tile
odd). ACT writes sigmoid outputs with stride-2 APs; the readout matmuls
read stride-2 views of z; dm is pair-duplicated on the host.

Readout: m_num_t = sum_n d_n s_tn = d16sum*(2*M_t - 1) - 2*sum_n d_n z_tn
via K=128 fp16 readout matmuls accumulated over the 6 hysteron tiles;
the host applies the affine fixup and epilogue.

Sharding: hysteron dim N=5151 split across 8 cores (644 each, padded to
6 tiles of 128 partitions; padding carries density 0). Each core outputs
its readout partials [1, T]; host reduces across cores.
"""

import copy
import sys

import numpy as np

sys.path.insert(0, "/opt/trn_rl_repo")

N = 5151
T = 4096
TEMP = 0.01
NCORES = 8
P = 128
TILES = 6                 # ceil(644/128); per-core rows padded to 768
NPC = 644                 # hysterons per core (8*644 = 5152 >= N)
ROWS = TILES * P          # 768
K6 = 6                    # arg matmul contraction: a_hi,a_lo,b_hi,b_lo,1,1
PAIRS = TILES // 2        # 3 interleaved tile-pairs
T2 = 2 * T                # interleaved stream length per pair
CH = 512                  # readout chunk along t (one PSUM bank fp32)
ACH = 1024                # ACT/matmul chunk along t (per tile-half)
SCH2 = 4096               # scan chunk (interleaved elems; 2048 t-steps)
NSCH = T2 // SCH2         # 2 waves per pair
SCHT = SCH2 // 2          # t-steps per wave

_PROG_CACHE = {}


# --------------------------------------------------------------------------
# TTS2I: custom DVE uop -- two interleaved tensor_tensor_scans at
# ~1 elem/cycle.
#   out[p,2k]   = stA' = (stA + in0[p,2k])   * in1[p,2k]
#   out[p,2k+1] = stB' = (stB + in0[p,2k+1]) * in1[p,2k+1]
# stA init = s0 (C0, per-partition), stB init = s1 (C1).
# --------------------------------------------------------------------------

def _ref_tts2i(in0, in1, c0, c1, c2):
    d = np.asarray(in0, np.float32)
    a = np.asarray(in1, np.float32)
    Pn = d.shape[0]
    d = d.reshape(Pn, -1)
    a = a.reshape(Pn, -1)
    Nn = d.shape[1]

    def vec(c):
        if isinstance(c, np.ndarray):
            return np.broadcast_to(c.reshape(-1), (Pn,)).astype(
                np.float32).copy()
        return np.full((Pn,), c, np.float32)

    sA, sB = vec(c0), vec(c1)
    out = np.empty((Pn, Nn), np.float32)
    for k in range(Nn // 2):
        sA = (sA + d[:, 2 * k]) * a[:, 2 * k]
        out[:, 2 * k] = sA
        sB = (sB + d[:, 2 * k + 1]) * a[:, 2 * k + 1]
        out[:, 2 * k + 1] = sB
    return out


_TTS2I_CACHE = {}


class _Tts2iOp:
    """Duck-types the DveOp interface consumed by _custom_dve and
    dve_table_for_ops, but compiles to hand-built uops."""

    name = "TTS2I"
    subdim = False

    def __init__(self):
        from concourse.dve_spec import AluOp, C0, C1, Spec, Src0, Src1, scan
        self.spec = Spec(
            body=scan(AluOp.ADD, Src0 * Src1, init=C0) + C1 * Src1,
            reference=_ref_tts2i)

    def compile(self, ver):
        if ver in _TTS2I_CACHE:
            return _TTS2I_CACHE[ver]
        assert ver == "v3", "TTS2I authored for TRN2 (v3) only"
        from concourse import dve_ops
        from concourse.dve_spec import AluOp, C0, Spec, Src0, Src1, lower, scan
        from concourse.dve_uop import (AluInp, DelayInp, DveOpSpec, InpSel,
                                       Trigger)

        base = lower(Spec(body=scan(AluOp.ADD, Src0 * Src1, init=C0)),
                     ver=ver)
        AluOpDp = type(base[0].datapath_config[0].op)

        p1 = copy.deepcopy(base[0])
        p2 = copy.deepcopy(base[0])
        steady_a = copy.deepcopy(base[1])

        for u in (p1, p2, steady_a):
            # add CONST_1 on input lane 4 -> delay lane 3
            u.inp[4] = InpSel.CONST_1
            u.inp_enable[4] = 1
            for blk in u.datapath_config:
                blk.delay[3] = DelayInp.PREV_DELAY
                blk.delay_enable[3] = 1

        # Primers: P1 latches C0 into stage1's a-flop, P2 latches C1 into
        # the b-flop. Separate flops survive pipeline bubbles between the
        # primers and the first data elements.
        for p, lane, a_en, b_en in (
            (p1, AluInp.PREV_DELAY_2, 1, 0),
            (p2, AluInp.PREV_DELAY_3, 0, 1),
        ):
            s0b = p.datapath_config[0]
            s0b.op = AluOpDp.BYPASS
            s0b.alu_src0 = AluInp.PREV_DELAY_0
            s0b.alu_src1 = AluInp.PREV_DELAY_0
            s1b = p.datapath_config[1]
            s1b.op = AluOpDp.BYPASS
            s1b.alu_src0 = lane
            s1b.alu_src1 = lane
            s1b.alu_out_enable = 1
            s1b.alu_out_a_enable = a_en
            s1b.alu_out_b_enable = b_en
        p1.trigger = (Trigger.COUNT, Trigger.NONE, Trigger.NONE)
        p1.repeat_count = 1
        p1.next_uop = (1, 0, 0)
        p2.trigger = (Trigger.COUNT, Trigger.NONE, Trigger.NONE)
        p2.repeat_count = 1
        p2.next_uop = (2, 0, 0)

        # Steady state: alternating uops; stream A reads/writes the a-flop,
        # stream B the b-flop.
        steady_b = copy.deepcopy(steady_a)
        for u, flop, a_en, b_en, nxt in (
            (steady_a, AluInp.NEXT_ALU_OUT_A, 1, 0, 3),
            (steady_b, AluInp.NEXT_ALU_OUT_B, 0, 1, 2),
        ):
            s0b = u.datapath_config[0]
            s0b.op = AluOpDp.ADD
            s0b.alu_src0 = AluInp.PREV_DELAY_0
            s0b.alu_src1 = flop
            s1b = u.datapath_config[1]
            s1b.op = AluOpDp.MULTIPLY
            s1b.alu_src0 = AluInp.PREV_ALU_OUT
            s1b.alu_src1 = AluInp.PREV_DELAY_1
            s1b.alu_out_enable = 1
            s1b.alu_out_a_enable = a_en
            s1b.alu_out_b_enable = b_en
            u.trigger = (Trigger.SRC_TENSOR_DONE, Trigger.COUNT, Trigger.NONE)
            u.repeat_count = 1
            u.next_uop = (0, nxt, 0)

        spec = DveOpSpec(
            name=self.name,
            opcode=dve_ops.get_dve_sub_opcode(self.name),
            uops=[p1, p2, steady_a, steady_b],
            rd1_en=True,
        )
        _TTS2I_CACHE[ver] = spec
        return spec


def _register_tts2i():
    from concourse import dve_ops
    if "TTS2I" in dve_ops._SUB_OPCODE_FOR_NAME:
        return next(o for o in dve_ops.OPS if o.name == "TTS2I")
    op = _Tts2iOp()
    row = max(dve_ops._SUB_OPCODE_FOR_NAME.values()) + 1
    assert row < 0x20
    dve_ops._SUB_OPCODE_FOR_NAME["TTS2I"] = row
    dve_ops.OPS.append(op)
    dve_ops.CUSTOM_DVE_SPECS["TTS2I"] = op.spec
    return op


# --------------------------------------------------------------------------
# Program
# --------------------------------------------------------------------------

def _build_program(reps=1, loop_n=0, skip=()):
    import contextlib

    import concourse.bass as bass
    import concourse.tile as tile
    from concourse import bacc, mybir

    tts2i = _register_tts2i()

    f32 = mybir.dt.float32
    f16 = mybir.dt.float16
    nc = bacc.Bacc("TRN2", target_bir_lowering=False, debug=False,
                   num_devices=NCORES)

    wt_d = nc.dram_tensor("wt", [K6, ROWS], f16, kind="ExternalInput")
    v_d = nc.dram_tensor("v", [K6, T], f16, kind="ExternalInput")
    dm_d = nc.dram_tensor("dm", [T], f16, kind="ExternalInput")
    dens_d = nc.dram_tensor("dens", [P, TILES], f16, kind="ExternalInput")
    mpart_d = nc.dram_tensor("mpart", [1, T], f32, kind="ExternalOutput")

    wt_ap = wt_d.ap()
    v_ap = v_d.ap()
    dm_ap = dm_d.ap()
    dens_ap = dens_d.ap()
    mpart_ap = mpart_d.ap()

    ts = bass.ts
    Sigmoid = mybir.ActivationFunctionType.Sigmoid

    def strided(base, off, step, cnt):
        # [128, cnt] view of `base` (a full-tile AP) at inner offset `off`
        # with inner stride `step`
        return bass.AP(tensor=base.tensor, offset=base.offset + off,
                       ap=[list(base.ap[0]), [step, cnt]])

    with tile.TileContext(nc) as tc:
        from contextlib import ExitStack
        with ExitStack() as ctx:
            consts = ctx.enter_context(tc.tile_pool(name="consts", bufs=1))
            apool = ctx.enter_context(tc.tile_pool(name="a", bufs=3))
            zpool = ctx.enter_context(tc.tile_pool(name="z", bufs=PAIRS))
            mpool = ctx.enter_context(tc.tile_pool(name="m", bufs=2))
            ps_arg = ctx.enter_context(
                tc.tile_pool(name="ps_arg", bufs=3, space="PSUM"))
            ps_m = ctx.enter_context(
                tc.tile_pool(name="ps_m", bufs=2, space="PSUM"))

            wt_sb = consts.tile([K6, ROWS], f16)
            v_sb = consts.tile([K6, T], f16)
            dens_sb = consts.tile([P, TILES], f16)
            dm_bc = consts.tile([P, T], f16)
            ini32 = consts.tile([P, 2 * PAIRS], f32)

            nc.sync.dma_start(out=wt_sb[:], in_=wt_ap[:, :])
            nc.sync.dma_start(out=v_sb[:], in_=v_ap[:, :])
            # broadcast the dM row to all 128 partitions via 0-stride DMA,
            # chunked across queues; the scan reads it pair-duplicated
            # through a stride-0 inner AP dim
            for j in range(8):
                c = T // 8
                src = bass.AP(tensor=dm_ap.tensor,
                              offset=dm_ap.offset + j * c,
                              ap=[[0, P], [1, c]])
                nc.sync.dma_start(out=dm_bc[:, ts(j, c)], in_=src)
            nc.sync.dma_start(out=dens_sb[:], in_=dens_ap[:, :])

            if loop_n:
                loop_cm = tc.For_i(
                    0, loop_n, 1,
                    hint_engines=(mybir.EngineType.PE,
                                  mybir.EngineType.Activation,
                                  mybir.EngineType.DVE,
                                  mybir.EngineType.Pool))
            else:
                loop_cm = contextlib.nullcontext()
            with loop_cm:
              for _rep in range(reps):
                z_tiles = [zpool.tile([P, T2], f16, name=f"z{p}")
                           for p in range(PAIRS)]

                def emit_readout(c):
                    # readout for the t-range covered by wave c; emitted one
                    # wave late so the in-order PE queue never stalls the
                    # next wave's arg matmuls behind scan-dependent readouts
                    m_sb = mpool.tile([1, SCHT], f32, tag="m")
                    for jj in range(SCHT // CH):
                        t0 = c * SCHT + jj * CH
                        mp = ps_m.tile([1, CH], f32)
                        for p in range(PAIRS):
                            zbase = z_tiles[p][:]
                            for half in range(2):
                                i = 2 * p + half
                                rhs = strided(zbase, 2 * t0 + half, 2, CH)
                                nc.tensor.matmul(
                                    out=mp[:],
                                    lhsT=dens_sb[:, i:i + 1],
                                    rhs=rhs,
                                    start=(p == 0 and half == 0),
                                    stop=(p == PAIRS - 1 and half == 1),
                                )
                        nc.scalar.copy(out=m_sb[:, ts(jj, CH)], in_=mp[:])
                        nc.sync.dma_start(
                            out=mpart_ap[:, t0:t0 + CH],
                            in_=m_sb[:, ts(jj, CH)])

                for c in range(NSCH):
                    for p in range(PAIRS):
                        a = apool.tile([P, SCH2], f16)
                        for half in range(2):
                            i = 2 * p + half       # hysteron tile index
                            for aj in range(SCHT // ACH):
                                arg = ps_arg.tile([P, ACH], f32, tag="arg")
                                for jj in range(ACH // CH):
                                    j0 = c * SCHT + aj * ACH + jj * CH
                                    nc.tensor.matmul(
                                        out=arg[:, ts(jj, CH)],
                                        lhsT=wt_sb[:, ts(i, P)],
                                        rhs=v_sb[:, j0:j0 + CH],
                                        start=True, stop=True,
                                    )
                                # a[2k+half] = sigmoid(-arg), strided write
                                dst = strided(a[:], 2 * aj * ACH + half,
                                              2, ACH)
                                nc.scalar.activation(
                                    out=dst, in_=arg[:],
                                    func=Sigmoid, scale=-1.0)
                        if "scan" not in skip:
                            off = c * SCH2
                            if c == 0:
                                iniA, iniB = 0.0, 0.0
                            else:
                                # ISA wants fp32 scalar APs; stage the z
                                # chain values through a tiny fp32 copy
                                nc.vector.tensor_copy(
                                    out=ini32[:, 2 * p:2 * p + 2],
                                    in_=z_tiles[p][:, off - 2:off])
                                iniA = ini32[:, 2 * p:2 * p + 1]
                                iniB = ini32[:, 2 * p + 1:2 * p + 2]
                            dmb = dm_bc[:]
                            dm_in = bass.AP(
                                tensor=dmb.tensor,
                                offset=dmb.offset + c * SCHT,
                                ap=[list(dmb.ap[0]), [1, SCHT], [0, 2]])
                            nc.vector._custom_dve(
                                tts2i,
                                out=z_tiles[p][:, ts(c, SCH2)],
                                in0=dm_in,
                                in1=a[:],
                                s0=iniA, s1=iniB)
                        else:
                            nc.vector.tensor_copy(
                                out=z_tiles[p][:, ts(c, SCH2)], in_=a[:])

                    if c >= 1:
                        emit_readout(c - 1)
                emit_readout(NSCH - 1)
    nc.compile()
    return nc


def _split16(x):
    hi = x.astype(np.float16)
    lo = (x - hi.astype(np.float64)).astype(np.float16)
    return hi, lo


def _host_prep(h, mesh_points, raw_density):
    h = np.asarray(h, np.float32)
    mesh = np.asarray(mesh_points, np.float32)
    rd = np.asarray(raw_density, np.float32)
    beta = mesh[:, 0].astype(np.float64)
    alpha = mesh[:, 1].astype(np.float64)

    hprev = np.concatenate([[np.float32(0.0)], h[:-1]])
    up = h > hprev
    R = np.float64(1.0) / np.float64(np.float32(TEMP))
    h64 = h.astype(np.float64)
    q = np.where(up, -R, 0.0)
    r = np.where(up, 0.0, R)
    p = np.where(up, R * h64, -R * h64)
    p_hi, p_lo = _split16(p)
    q16 = q.astype(np.float16)
    r16 = r.astype(np.float16)
    V6 = np.stack([q16, q16, r16, r16, p_hi, p_lo]).astype(np.float16)

    M = up.astype(np.float64)                 # M_t in {0,1}
    Mprev = np.concatenate([[0.0], M[:-1]])
    dM = (M - Mprev).astype(np.float16)       # in {-1,0,1}

    dens = (1.0 / (1.0 + np.exp(-rd.astype(np.float64))))  # [N] float64

    pad = NCORES * NPC - N   # 1
    alpha_p = np.concatenate([alpha, np.full(pad, 0.5)])
    beta_p = np.concatenate([beta, np.full(pad, 0.5)])
    dens_p = np.concatenate([dens, np.zeros(pad)])

    in_maps = []
    d16sum = 0.0
    for c in range(NCORES):
        sl = slice(c * NPC, (c + 1) * NPC)
        a_c = np.full(ROWS, 0.5)
        b_c = np.full(ROWS, 0.5)
        d_c = np.zeros(ROWS)
        a_c[:NPC] = alpha_p[sl]
        b_c[:NPC] = beta_p[sl]
        d_c[:NPC] = dens_p[sl]
        ah, al = _split16(a_c)
        bh, bl = _split16(b_c)
        wt = np.stack([ah, al, bh, bl,
                       np.ones(ROWS, np.float16),
                       np.ones(ROWS, np.float16)]).astype(np.float16)
        dens16 = d_c.astype(np.float16)
        dens_tiles = dens16.reshape(TILES, P).T  # [P, TILES]
        d16sum += dens16.astype(np.float64).sum()
        in_maps.append({
            "wt": wt,
            "v": V6,
            "dm": dM,
            "dens": dens_tiles,
        })
    return in_maps, dens, h, d16sum, M


def kernel(h, mesh_points, raw_density, raw_offset, raw_scale, raw_slope):
    from concourse.bass_utils import run_bass_kernel_spmd

    in_maps, dens, h32, d16sum, M = _host_prep(h, mesh_points, raw_density)

    if "prog" not in _PROG_CACHE:
        _PROG_CACHE["prog"] = _build_program()
    nc = _PROG_CACHE["prog"]

    res = run_bass_kernel_spmd(nc, in_maps, list(range(NCORES)))
    zpart = np.zeros(T, np.float64)
    for c in range(NCORES):
        zpart += res.results[c]["mpart"].astype(np.float64).reshape(T)

    def sigm(x):
        return 1.0 / (1.0 + np.exp(-np.float64(np.asarray(x, np.float32)[0])))

    offset = -10.0 + 20.0 * sigm(raw_offset)
    scale = 20.0 * sigm(raw_scale)
    slope = -20.0 + 40.0 * sigm(raw_slope)

    # s = 2u-1, u = M - z  =>  sum(d*s) = d16sum*(2M-1) - 2*sum(d*z)
    m = (d16sum * (2.0 * M - 1.0) - 2.0 * zpart) / dens.sum()
    out = scale * m + h32.astype(np.float64) * slope + offset
    return out.astype(np.float32)
